# revision 1
# baseline (speedup 1.0000x reference)
"""GQA self-attention block (q/k/v proj + causal softmax attention + o proj)
on 8 trn2 NeuronCores.

Sharding: batch (2) x query-head-groups (4) -> 8 cores. Core c handles
batch b=c//4 and heads [8g, 8g+8) where g=c%4 (kv heads [2g, 2g+2)).
Each core computes a partial output [T, D] = ctx_heads @ o_proj_cols.T;
the host sums the 4 partials per batch (all-reduce done host-side).

All matmuls run as float32r (fp32 rounded to 11-bit mantissa by the
producer, exact fp32 accumulation in PSUM) which streams at 1 cycle/row
for moving-dim >= 256.

Layout strategy (per core), everything d-major so no on-chip transposes of
x are needed (host passes x.T / weights pre-transposed):
  phase 1: qkvT[j, t] = wqkvT.T @ xT  (j = packed q|k|v output dims)
  phase 2: per head pair, S.T[j_keys, i_queries] = kT.T @ qT tiles
           (K=64 row-tiled at partitions 0/64 -> both heads concurrent),
           E = exp((S + mask)/8) on ACT, ctxT[e, i] accumulated as
           v_plus.T @ E with an appended ones column giving the softmax
           denominator in row 64; normalize with DVE reciprocal +
           PE broadcast + DVE multiply.
  phase 3: out[t, r] = ctxT.T @ o_projT, PSUM DMA'd straight to DRAM.
"""

import os
import sys

sys.path.insert(0, "/opt/trn_rl_repo")

import numpy as np

import concourse.bass as bass
import concourse.tile as tile
from concourse import bacc, mybir
from concourse.bass_utils import run_bass_kernel_spmd

F32 = mybir.dt.float32
F32R = mybir.dt.float32r
EXP = mybir.ActivationFunctionType.Exp

B, T, D = 2, 2048, 2048
HQ, HK = 32, 8
DH = D // HQ              # 64 head dim
N_CORES = 8
GROUPS = 4                # head groups per batch
QCOLS = D // GROUPS       # 512 q cols per core
KCOLS = (D // 4) // GROUPS  # 128 k cols per core (2 kv heads)
WCOLS = QCOLS + 2 * KCOLS   # 768
TB = 256                  # phase-1 token block
NTB = T // TB             # 8
KT = D // 128             # 16 contraction tiles
NEG = -480.0              # additive mask pre-scale (-60 after 1/8 scale)

_cache = {}


def _build():
    nc = bacc.Bacc("TRN2", target_bir_lowering=False, debug=False)

    xT_d = nc.declare_dram_parameter("xT", [D, T], F32R, isOutput=False)
    wqkv_d = nc.declare_dram_parameter("wqkv", [D, WCOLS], F32R, isOutput=False)
    oproj_d = nc.declare_dram_parameter("oproj", [QCOLS, D], F32R, isOutput=False)
    masks_d = nc.declare_dram_parameter("masks", [2, 128, 128], mybir.dt.bfloat16, isOutput=False)
    ident_d = nc.declare_dram_parameter("ident", [128, 64], F32R, isOutput=False)
    ones_d = nc.declare_dram_parameter("ones", [128, 16 * 65], F32R, isOutput=False)
    out_d = nc.declare_dram_parameter("out", [T, D], F32, isOutput=True)
    rcscr_d = nc.dram_tensor("rcscratch", [16, 1024], F32)

    with tile.TileContext(nc) as tc:
        with (
            tc.tile_pool(name="pers", bufs=1) as pers,
            tc.tile_pool(name="xt", bufs=24) as xtp,
            tc.tile_pool(name="work", bufs=2) as work,
            tc.tile_pool(name="psum", bufs=1, space="PSUM") as psum,
        ):
            # ---- constants / weights ----
            wqkv_sb = pers.tile([128, KT, WCOLS], F32R, tag="wslot")
            wq_r = wqkv_d[:].rearrange("(k p) c -> p k c", p=128)
            xt0 = []
            for k in range(KT):
                nc.sync.dma_start(
                    wqkv_sb[:, k, 0:128], wq_r[:, k, 0:128]
                )
                xt = xtp.tile([128, TB], F32R, tag="xt", bufs=24, name="xt0")
                nc.sync.dma_start(xt, xT_d[128 * k : 128 * k + 128, 0:TB])
                xt0.append(xt)
            for m in range(1, 6):
                csl = slice(128 * m, 128 * m + 128)
                for k in range(KT):
                    nc.sync.dma_start(wqkv_sb[:, k, csl], wq_r[:, k, csl])
            masks_sb = pers.tile([128, 2, 128], mybir.dt.bfloat16, tag="masks")
            ident_sb = pers.tile([128, 64], F32R, tag="ident")

            qt = [pers.tile([128, T], F32R, tag=f"qt{m}", name=f"qt{m}") for m in range(4)]
            kp = [pers.tile([128, T], F32R, tag=f"kp{k}", name=f"kp{k}") for k in range(2)]
            vT = pers.tile([128, T], F32R, tag="ctx0")
            vs = [pers.tile([128, 16 * 65], F32R, tag=f"vs{k}", name=f"vs{k}") for k in range(2)]
            ctx = [pers.tile([128, T], F32R, tag=f"ctx{m}", name=f"ctx{m}") for m in range(4)]

            nc.sync.dma_start(vs[0], ones_d[:])
            nc.sync.dma_start(vs[1], ones_d[:])

            # ---- phase 1: qkvT = wqkvT.T @ xT ----
            for tb in range(NTB):
                ts = slice(tb * TB, tb * TB + TB)
                if tb == 0:
                    xts = xt0
                else:
                    xts = []
                    for k in range(KT):
                        xt = xtp.tile([128, TB], F32R, tag="xt", bufs=24,
                                      name="xt")
                        nc.sync.dma_start(xt, xT_d[128 * k : 128 * k + 128, ts])
                        xts.append(xt)
                for m in range(6):
                    ps = psum.tile([128, TB], F32, tag="s2", bufs=4)
                    for k in range(KT):
                        nc.tensor.matmul(
                            ps,
                            wqkv_sb[:, k, 128 * m : 128 * m + 128],
                            xts[k],
                            start=(k == 0),
                            stop=(k == KT - 1),
                        )
                    if m < 4:
                        nc.scalar.copy(qt[m][:, ts], ps)
                    elif m == 4:
                        nc.scalar.copy(kp[0][0:64, ts], ps[0:64, :])
                        nc.scalar.copy(kp[1][64:128, ts], ps[64:128, :])
                    else:
                        nc.scalar.copy(vT[:, ts], ps)

            # constants needed from phase 1b on (emitted late so their DMAs
            # don't delay the phase-1 input stream)
            nc.sync.dma_start(
                masks_sb, masks_d[:].rearrange("o p f -> p o f")
            )
            nc.sync.dma_start(ident_sb, ident_d[:])

            # kT duplicates at the other partition half (SBUF->SBUF DMA
            # handles the partition shift)
            nc.sync.dma_start(kp[0][64:128, :], kp[0][0:64, :])
            nc.sync.dma_start(kp[1][0:64, :], kp[1][64:128, :])

            # ---- phase 1b: v = vT.T per 128-chunk, with ones column ----
            for kv in range(2):
                rows = slice(64 * kv, 64 * kv + 64)
                for c in range(16):
                    tp = psum.tile([128, 64], F32R, tag="s2", bufs=4)
                    nc.tensor.transpose(
                        tp,
                        vT[rows, 128 * c : 128 * c + 128],
                        ident_sb[rows, :],
                        tile_position=(64 * kv, 0),
                    )
                    nc.vector.tensor_copy(
                        vs[kv][:, 65 * c : 65 * c + 64], tp
                    )

            # ---- phase 2: attention per head pair ----
            # Per (head-pair m, query block a of 512): S.T pairs row-tiled at
            # partitions 0/64, exp on ACT, ctxT accumulated in PSUM with a
            # ones column giving the softmax denominator in row 64. Diagonal
            # 128-key chunks are trimmed to their valid query range with a
            # small [128,128] triangle mask. Normalization runs two blocks
            # deferred, on SBUF copies, so nothing ever waits on it.
            pending_norm = []
            for m in range(4):
                kv = m // 2
                for a in range(4):
                    nj = 4 * (a + 1)
                    isl = slice(512 * a, 512 * a + 512)
                    ctxAB = psum.tile([65, 1024], F32, tag="s2", bufs=4)
                    pend = []
                    for jc in range(nj):
                        if jc == 2 and len(pending_norm) >= 2:
                            pending_norm.pop(0)()
                        o = jc - 4 * a
                        lo = (0, 128, 256, 256)[o] if o >= 0 else 0
                        n = 512 - lo
                        jsl = slice(128 * jc, 128 * jc + 128)
                        S = psum.tile([128, 1024], F32, tag="s2", bufs=4)
                        for h2 in range(2):
                            nc.tensor.matmul(
                                S[:, 512 * h2 + lo : 512 * h2 + 512],
                                kp[kv][64 * h2 : 64 * h2 + 64, jsl],
                                qt[m][64 * h2 : 64 * h2 + 64,
                                      512 * a + lo : 512 * a + 512],
                                start=True,
                                stop=True,
                                tile_position=(64 * h2, 0),
                            )
                        if o >= 0:
                            tri = 128 * o  # triangle start col
                            for h2 in range(2):
                                base = 512 * h2
                                nc.vector.tensor_add(
                                    S[:, base + tri : base + tri + 128],
                                    S[:, base + tri : base + tri + 128],
                                    masks_sb[:, 0, :],
                                )
                                if o == 3:
                                    nc.vector.tensor_add(
                                        S[:, base + 256 : base + 384],
                                        S[:, base + 256 : base + 384],
                                        masks_sb[:, 1, :],
                                    )
                        E = work.tile([128, 1024], F32R, tag="E", bufs=4)
                        if lo == 0:
                            nc.scalar.activation(E, S, EXP, scale=0.125)
                        else:
                            for h2 in range(2):
                                base = 512 * h2
                                nc.scalar.activation(
                                    E[:, base + lo : base + 512],
                                    S[:, base + lo : base + 512],
                                    EXP,
                                    scale=0.125,
                                )
                        pend.append((E, jc, lo))
                        if len(pend) > 2:
                            pE, pjc, plo = pend.pop(0)
                            for h2 in range(2):
                                base = 512 * h2
                                nc.tensor.matmul(
                                    ctxAB[:, base + plo : base + 512],
                                    vs[kv][:, 65 * pjc : 65 * pjc + 65],
                                    pE[:, base + plo : base + 512],
                                    start=(pjc == 0),
                                    stop=False,
                                )
                    while pend:
                        pE, pjc, plo = pend.pop(0)
                        for h2 in range(2):
                            base = 512 * h2
                            nc.tensor.matmul(
                                ctxAB[:, base + plo : base + 512],
                                vs[kv][:, 65 * pjc : 65 * pjc + 65],
                                pE[:, base + plo : base + 512],
                                start=(pjc == 0),
                                stop=(pjc == nj - 1),
                            )

                    # one fast copy frees the ctx PSUM slot; the rest of the
                    # normalize chain runs two blocks later on the SBUF copy
                    cu = work.tile([65, 1024], F32, tag="cu", bufs=3,
                                   name="cu")
                    nc.vector.tensor_copy(cu, ctxAB)

                    def _normalize(cu=cu, m=m, a=a, isl=isl):
                        den128 = work.tile([128, 8], F32, tag="d128", bufs=2,
                                           name="den128")
                        nc.sync.dma_start(den128, cu[64:65, :])
                        rcp = work.tile([128, 8], F32, tag="rcp", bufs=2,
                                        name="rcp")
                        nc.vector.reciprocal(rcp, den128)
                        ma = m * 4 + a
                        nc.sync.dma_start(rcscr_d[ma : ma + 1, :], rcp)
                        bcs = work.tile([64, 1024], F32, tag="bcs", bufs=2,
                                        name="bcs")
                        nc.sync.dma_start(
                            bcs, rcscr_d[ma : ma + 1, :].partition_broadcast(64)
                        )
                        nc.vector.tensor_mul(
                            ctx[m][0:64, isl], cu[0:64, 0:512], bcs[:, 0:512]
                        )
                        tmpB = work.tile([64, 512], F32R, tag="tb", bufs=2)
                        nc.vector.tensor_mul(
                            tmpB, cu[0:64, 512:1024], bcs[:, 512:1024]
                        )
                        nc.sync.dma_start(ctx[m][64:128, isl], tmpB)

                    pending_norm.append(_normalize)

            while pending_norm:
                pending_norm.pop(0)()

            # ---- phase 3: out = ctxT.T @ o_projT ----
            oproj_sb = pers.tile([128, 4, D], F32R, tag="wslot")
            nc.sync.dma_start(
                oproj_sb, oproj_d[:].rearrange("(m p) c -> p m c", p=128)
            )
            for t in range(16):
                tsl = slice(128 * t, 128 * t + 128)
                for rp in range(2):
                    ps = psum.tile([128, 1024], F32, tag="s2", bufs=4)
                    for m in range(4):
                        for half in range(2):
                            r = 2 * rp + half
                            nc.tensor.matmul(
                                ps[:, 512 * half : 512 * half + 512],
                                ctx[m][:, tsl],
                                oproj_sb[:, m, 512 * r : 512 * r + 512],
                                start=(m == 0),
                                stop=(m == 3),
                            )
                    ostage = work.tile([128, 1024], F32, tag="E", bufs=4,
                                       name="ostage")
                    nc.scalar.copy(ostage, ps)
                    nc.sync.dma_start(
                        out_d[tsl, 1024 * rp : 1024 * rp + 1024], ostage
                    )

    nc.compile()
    return nc


def _host_inputs(x, q_proj, k_proj, v_proj, o_proj):
    """Per-core input dicts (numpy, float32)."""
    masks = np.zeros((2, 128, 128), dtype=np.float32)
    jj = np.arange(128)[:, None]
    ii = np.arange(128)[None, :]
    masks[0] = np.where(jj <= ii, 0.0, NEG)
    masks[1] = NEG
    import ml_dtypes
    masks = masks.astype(ml_dtypes.bfloat16)
    ident = np.zeros((128, 64), dtype=np.float32)
    ident[np.arange(64), np.arange(64)] = 1.0
    ident[np.arange(64) + 64, np.arange(64)] = 1.0

    xT = [np.ascontiguousarray(x[b].T) for b in range(B)]
    in_maps = []
    for c in range(N_CORES):
        b, g = divmod(c, GROUPS)
        wqkv = np.concatenate(
            [
                q_proj[QCOLS * g : QCOLS * g + QCOLS].T,
                k_proj[KCOLS * g : KCOLS * g + KCOLS].T,
                v_proj[KCOLS * g : KCOLS * g + KCOLS].T,
            ],
            axis=1,
        )
        in_maps.append(
            {
                "xT": xT[b],
                "wqkv": np.ascontiguousarray(wqkv),
                "oproj": np.ascontiguousarray(
                    o_proj[:, QCOLS * g : QCOLS * g + QCOLS].T
                ),
                "masks": masks,
                "ident": ident,
                "ones": np.ones((128, 16 * 65), dtype=np.float32),
            }
        )
    return in_maps


def run(x, q_proj, k_proj, v_proj, o_proj, trace=False):
    """Run on hardware; returns (output [B,T,D] f32, BassKernelResults)."""
    if "nc" not in _cache:
        _cache["nc"] = _build()
    nc = _cache["nc"]
    in_maps = _host_inputs(x, q_proj, k_proj, v_proj, o_proj)
    res = run_bass_kernel_spmd(
        nc, in_maps, core_ids=list(range(N_CORES)), trace=trace
    )
    parts = [res.results[c]["out"] for c in range(N_CORES)]
    out = np.empty((B, T, D), dtype=np.float32)
    for b in range(B):
        acc = parts[4 * b].astype(np.float64)
        for g in range(1, GROUPS):
            acc += parts[4 * b + g]
        out[b] = acc.astype(np.float32)
    return out, res


def kernel(x, q_proj, k_proj, v_proj, o_proj, hq=None, hk=None, **_unused):
    x = np.asarray(x, dtype=np.float32)
    q_proj = np.asarray(q_proj, dtype=np.float32)
    k_proj = np.asarray(k_proj, dtype=np.float32)
    v_proj = np.asarray(v_proj, dtype=np.float32)
    o_proj = np.asarray(o_proj, dtype=np.float32)
    assert x.shape == (B, T, D), x.shape
    trace = bool(os.environ.get("KERNEL_TRACE"))
    out, _ = run(x, q_proj, k_proj, v_proj, o_proj, trace=trace)
    return out



# revision 9
# speedup vs baseline: 1.3706x; 1.3706x over previous
"""GQA self-attention block (q/k/v proj + causal softmax attention + o proj)
on 8 trn2 NeuronCores.

Sharding: batch (2) x query-head-groups (4) -> 8 cores. Core c handles
batch b=c//4 and heads [8g, 8g+8) where g=c%4 (kv heads [2g, 2g+2)).
Each core computes a partial transposed output [D, T] = o_proj_cols @ ctx;
the host sums the 4 partials per batch and transposes (all-reduce host-side).

v2 design notes (from perfetto trace of the fp32r baseline: PE stream 255us
but PE-array occupancy 437us -> ~180us of unhidden stationary loads, phase-2
at 35% stream efficiency, phases fully serial):
  - everything bf16 (same 1 cycle/row PE stream rate as fp32r, but half the
    stationary-load time, half the DMA bytes, no moving>=256 restriction so
    the causal diagonal is trimmed to 128-col granularity).
  - phase 1 streams 1024-token blocks per weight load (16-matmul PSUM
    accumulation chains hide the loads entirely).
  - phase 2 keeps keys-stationary chunks but emits the two per-chunk ctx
    matmuls as one strided-AP matmul, exp as one strided ACT op.
  - phase 3 computes out.T (o_proj cols stationary, ctx moving) so each
    stationary load streams 1024 tokens; host transposes.
  - phases interleaved for engine overlap: phase-2 rounds for query blocks
    a=0,1 run between phase-1 groups (ACT exp overlaps PE projections);
    phase-1 second half runs inside the a=0,1 round stream; phase-3 first
    token half is interleaved into the a=2,3 rounds (fills PE while ACT
    does exp); only the second token half of phase 3 trails.
  - v transposed via XBAR dma_start_transpose (no PE/PSUM involvement).
"""

import os
import sys

sys.path.insert(0, "/opt/trn_rl_repo")

import numpy as np

import concourse.bass as bass
import concourse.tile as tile
from concourse import bacc, mybir
from concourse.bass_utils import run_bass_kernel_spmd

F32 = mybir.dt.float32
BF16 = mybir.dt.bfloat16
EXP = mybir.ActivationFunctionType.Exp

B, T, D = 2, 2048, 2048
HQ, HK = 32, 8
DH = D // HQ              # 64 head dim
N_CORES = 8
GROUPS = 4                # head groups per batch
QCOLS = D // GROUPS       # 512 q cols per core
KCOLS = (D // 4) // GROUPS  # 128 k cols per core (2 kv heads)
WCOLS = QCOLS + 2 * KCOLS   # 768
KT = D // 128             # 16 contraction tiles
TB = 1024                 # phase-1/3 token block (half of T)
NEG = -480.0              # additive mask pre-scale (-60 after 1/8 scale)

_cache = {}


def _build():
    nc = bacc.Bacc("TRN2", target_bir_lowering=False, debug=False)

    xT_d = nc.declare_dram_parameter("xT", [D, T], BF16, isOutput=False)
    wqkv_d = nc.declare_dram_parameter("wqkv", [D, WCOLS], BF16, isOutput=False)
    oproj_d = nc.declare_dram_parameter("oproj", [128, 4, D], BF16, isOutput=False)
    masks_d = nc.declare_dram_parameter("masks", [128, 2, 128], BF16, isOutput=False)
    ones_d = nc.declare_dram_parameter("ones", [128, 16 * 80], BF16, isOutput=False)
    out_d = nc.declare_dram_parameter("out", [D, T], BF16, isOutput=True)
    rcscr_d = nc.dram_tensor("rcscratch", [16, 1024], F32)

    with tile.TileContext(nc) as tc:
        with (
            tc.tile_pool(name="pers", bufs=1) as pers,
            tc.tile_pool(name="xt", bufs=20) as xtp,
            tc.tile_pool(name="ep", bufs=4) as epool,
            tc.tile_pool(name="work", bufs=2) as work,
            tc.tile_pool(name="psum", bufs=1, space="PSUM") as psum,
        ):
            # ---- persistent SBUF ----
            wqkv_sb = pers.tile([128, KT, WCOLS], BF16, tag="wqkv")
            oproj_sb = pers.tile([128, 4, D], BF16, tag="oproj")
            masks_sb = pers.tile([128, 2, 128], BF16, tag="masks")
            qt = [pers.tile([128, T], BF16, tag=f"qt{m}", name=f"qt{m}")
                  for m in range(4)]
            kp = [pers.tile([128, T], BF16, tag=f"kp{k}", name=f"kp{k}")
                  for k in range(2)]
            vT = pers.tile([128, T], BF16, tag="vT")
            vs = [pers.tile([128, 16 * 80], BF16, tag=f"vs{k}", name=f"vs{k}")
                  for k in range(2)]
            ctx = [pers.tile([128, T], BF16, tag=f"ctx{m}", name=f"ctx{m}")
                   for m in range(4)]

            # ---- constant / weight DMAs ----
            nc.sync.dma_start(masks_sb, masks_d[:])
            nc.sync.dma_start(vs[0], ones_d[:])
            nc.sync.dma_start(vs[1], ones_d[:])
            wq_r = wqkv_d[:].rearrange("(k p) c -> p k c", p=128)
            for k in range(KT):
                nc.sync.dma_start(wqkv_sb[:, k, :], wq_r[:, k, :])

            xts = {}

            def load_x_half(half):
                ts = slice(TB * half, TB * half + TB)
                tiles = []
                for k in range(KT):
                    xt = xtp.tile([128, TB], BF16, tag="xt", bufs=20, name="xt")
                    nc.sync.dma_start(xt, xT_d[128 * k: 128 * k + 128, ts])
                    tiles.append(xt)
                xts[half] = tiles

            def v2(ap):
                return ap.rearrange("p (h q) -> p h q", h=2)

            # ---- phase 1: one 16-matmul chain per (m, token-half) ----
            def p1_group(mi, half, eng):
                ts = slice(TB * half, TB * half + TB)
                ps = psum.tile([128, TB], F32, tag="s2", bufs=3, name="p1ps")
                # matmul out must stay within one PSUM bank (512 f32 cols)
                for sub in (0, 512):
                    for k in range(KT):
                        nc.tensor.matmul(
                            ps[:, sub: sub + 512],
                            wqkv_sb[:, k, 128 * mi: 128 * mi + 128],
                            xts[half][k][:, sub: sub + 512],
                            start=(k == 0),
                            stop=(k == KT - 1),
                        )
                copy = eng.copy if eng is nc.scalar else eng.tensor_copy
                if mi < 4:
                    copy(qt[mi][:, ts], ps)
                elif mi == 4:
                    copy(kp[0][0:64, ts], ps[0:64, :])
                    copy(kp[1][64:128, ts], ps[64:128, :])
                    # kT duplicates at the other partition half
                    nc.sync.dma_start(kp[0][64:128, ts], kp[0][0:64, ts])
                    nc.sync.dma_start(kp[1][0:64, ts], kp[1][64:128, ts])
                else:
                    copy(vT[:, ts], ps)
                    # v chunks transposed into vs via XBAR dma (ones col 64
                    # of each 65-block left intact -> softmax denominator)
                    for kv in range(2):
                        vsr = vs[kv][:].rearrange("p (c e) -> p c e", e=80)
                        for c in range(8):
                            cc = 8 * half + c
                            nc.sync.dma_start_transpose(
                                vsr[:, cc, 0:64],
                                vT[64 * kv: 64 * kv + 64,
                                   128 * cc: 128 * cc + 128],
                            )

            # ---- phase 2: attention round per (head pair m, 512-query a) ----
            pending_norm = []

            def flush_norm(keep=0):
                while len(pending_norm) > keep:
                    pending_norm.pop(0)()

            def p2_round(m, a):
                kv = m // 2
                nj = 4 * (a + 1)
                qb = 512 * a
                ctxAB = psum.tile([65, 1024], F32, tag="ctx", bufs=1,
                                  name="ctxAB")
                pend = []

                def drain(last):
                    pE, pjc, plo = pend.pop(0)
                    for h2 in range(2):
                        nc.tensor.matmul(
                            ctxAB[:, 512 * h2 + plo: 512 * h2 + 512],
                            vs[kv][:, 80 * pjc: 80 * pjc + 65],
                            pE[:, 512 * h2 + plo: 512 * h2 + 512],
                            start=(pjc == 0),
                            stop=last,
                        )

                for jc in range(nj):
                    o = jc - 4 * a
                    lo = (0, 128, 256, 384)[o] if o >= 0 else 0
                    if jc == 2:
                        flush_norm(2)
                    S = psum.tile([128, 1024], F32, tag="s2", bufs=3, name="S")
                    for h2 in range(2):
                        nc.tensor.matmul(
                            S[:, 512 * h2 + lo: 512 * h2 + 512],
                            kp[kv][64 * h2: 64 * h2 + 64,
                                   128 * jc: 128 * jc + 128],
                            qt[m][64 * h2: 64 * h2 + 64, qb + lo: qb + 512],
                            start=True,
                            stop=True,
                            tile_position=(64 * h2, 0),
                        )
                    if o >= 0:
                        nc.vector.tensor_add(
                            v2(S)[:, :, lo: lo + 128],
                            v2(S)[:, :, lo: lo + 128],
                            masks_sb,
                        )
                    E = epool.tile([128, 1024], BF16, tag="E", bufs=4, name="E")
                    if lo == 0:
                        nc.scalar.activation(E, S, EXP, scale=0.125)
                    else:
                        nc.scalar.activation(
                            v2(E)[:, :, lo:512], v2(S)[:, :, lo:512],
                            EXP, scale=0.125,
                        )
                    pend.append((E, jc, lo))
                    if len(pend) > 2:
                        drain(False)
                while pend:
                    drain(len(pend) == 1)

                cu = work.tile([65, 1024], F32, tag="cu", bufs=3, name="cu")
                nc.vector.tensor_copy(cu, ctxAB)

                def _norm(cu=cu, m=m, a=a):
                    isl = slice(512 * a, 512 * a + 512)
                    den128 = work.tile([128, 8], F32, tag="d128", bufs=2,
                                       name="den128")
                    nc.sync.dma_start(den128, cu[64:65, :])
                    rcp = work.tile([128, 8], F32, tag="rcp", bufs=2,
                                    name="rcp")
                    nc.vector.reciprocal(rcp, den128)
                    ma = m * 4 + a
                    nc.sync.dma_start(rcscr_d[ma: ma + 1, :], rcp)
                    bcs = work.tile([64, 1024], F32, tag="bcs", bufs=2,
                                    name="bcs")
                    nc.sync.dma_start(
                        bcs, rcscr_d[ma: ma + 1, :].partition_broadcast(64)
                    )
                    nc.vector.tensor_mul(
                        ctx[m][0:64, isl], cu[0:64, 0:512], bcs[:, 0:512]
                    )
                    tmpB = work.tile([64, 512], BF16, tag="tb", bufs=2,
                                     name="tmpB")
                    nc.vector.tensor_mul(
                        tmpB, cu[0:64, 512:1024], bcs[:, 512:1024]
                    )
                    nc.sync.dma_start(ctx[m][64:128, isl], tmpB)

                pending_norm.append(_norm)

            # ---- phase 3: out.T group per (128-outcol chunk, token half) ----
            def p3_group(rc, th):
                tsl = slice(TB * th, TB * th + TB)
                ps3 = psum.tile([128, TB], F32, tag="s2", bufs=3, name="p3ps")
                for sub in (0, 512):
                    for m in range(4):
                        nc.tensor.matmul(
                            ps3[:, sub: sub + 512],
                            oproj_sb[:, m, 128 * rc: 128 * rc + 128],
                            ctx[m][:, TB * th + sub: TB * th + sub + 512],
                            start=(m == 0),
                            stop=(m == 3),
                        )
                ostage = work.tile([128, TB], BF16, tag="ostage", bufs=3,
                                   name="ostage")
                nc.vector.tensor_copy(ostage, ps3)
                nc.sync.dma_start(out_d[128 * rc: 128 * rc + 128, tsl], ostage)

            # ================= emission =================
            load_x_half(0)
            p1_group(4, 0, nc.scalar)
            p1_group(5, 0, nc.scalar)
            p1_group(0, 0, nc.vector)
            p2_round(0, 0)
            p1_group(1, 0, nc.scalar)
            p2_round(1, 0)
            p1_group(2, 0, nc.vector)
            p2_round(2, 0)
            p1_group(3, 0, nc.scalar)
            p2_round(3, 0)

            nc.sync.dma_start(oproj_sb, oproj_d[:])
            load_x_half(1)
            p2_round(0, 1)
            p1_group(4, 1, nc.vector)
            p2_round(1, 1)
            p1_group(5, 1, nc.vector)
            p2_round(2, 1)
            p1_group(0, 1, nc.vector)
            p2_round(3, 1)
            p1_group(1, 1, nc.vector)
            p1_group(2, 1, nc.vector)
            p1_group(3, 1, nc.vector)

            flush_norm(0)
            p3c = 0
            for a in (2, 3):
                for m in range(4):
                    p2_round(m, a)
                    p3_group(p3c, 0)
                    p3c += 1
                    p3_group(p3c, 0)
                    p3c += 1
            flush_norm(0)
            for rc in range(16):
                p3_group(rc, 1)

    nc.compile()
    return nc


def _host_inputs(x, q_proj, k_proj, v_proj, o_proj):
    """Per-core input dicts (numpy, bf16)."""
    import ml_dtypes
    bf = ml_dtypes.bfloat16

    jj = np.arange(128)[:, None]
    cc = np.arange(128)[None, :]
    tri = np.where(jj <= cc, 0.0, NEG).astype(np.float32)
    masks = np.stack([tri, tri], axis=1).astype(bf)  # [128, 2, 128]
    ones = np.ones((128, 16 * 80), dtype=np.float32).astype(bf)

    xT = [np.ascontiguousarray(x[b].T).astype(bf) for b in range(B)]
    in_maps = []
    for c in range(N_CORES):
        b, g = divmod(c, GROUPS)
        wqkv = np.concatenate(
            [
                q_proj[QCOLS * g: QCOLS * g + QCOLS].T,
                k_proj[KCOLS * g: KCOLS * g + KCOLS].T,
                v_proj[KCOLS * g: KCOLS * g + KCOLS].T,
            ],
            axis=1,
        ).astype(bf)
        op = o_proj[:, QCOLS * g: QCOLS * g + QCOLS].T  # [512 e, 2048 r]
        op = np.ascontiguousarray(
            op.reshape(4, 128, D).transpose(1, 0, 2)
        ).astype(bf)
        in_maps.append(
            {
                "xT": xT[b],
                "wqkv": np.ascontiguousarray(wqkv),
                "oproj": op,
                "masks": masks,
                "ones": ones,
            }
        )
    return in_maps


def run(x, q_proj, k_proj, v_proj, o_proj, trace=False):
    """Run on hardware; returns (output [B,T,D] f32, BassKernelResults)."""
    if "nc" not in _cache:
        _cache["nc"] = _build()
    nc = _cache["nc"]
    in_maps = _host_inputs(x, q_proj, k_proj, v_proj, o_proj)
    res = run_bass_kernel_spmd(
        nc, in_maps, core_ids=list(range(N_CORES)), trace=trace
    )
    parts = [res.results[c]["out"] for c in range(N_CORES)]
    out = np.empty((B, T, D), dtype=np.float32)
    for b in range(B):
        acc = parts[4 * b].astype(np.float64)
        for g in range(1, GROUPS):
            acc += parts[4 * b + g].astype(np.float64)
        out[b] = acc.T.astype(np.float32)
    return out, res


def kernel(x, q_proj, k_proj, v_proj, o_proj, hq=None, hk=None, **_unused):
    x = np.asarray(x, dtype=np.float32)
    q_proj = np.asarray(q_proj, dtype=np.float32)
    k_proj = np.asarray(k_proj, dtype=np.float32)
    v_proj = np.asarray(v_proj, dtype=np.float32)
    o_proj = np.asarray(o_proj, dtype=np.float32)
    assert x.shape == (B, T, D), x.shape
    trace = bool(os.environ.get("KERNEL_TRACE"))
    out, _ = run(x, q_proj, k_proj, v_proj, o_proj, trace=trace)
    return out


# revision 11
# speedup vs baseline: 1.4094x; 1.0283x over previous
"""GQA self-attention block (q/k/v proj + causal softmax attention + o proj)
on 8 trn2 NeuronCores.

Sharding: batch (2) x query-head-groups (4) -> 8 cores. Core c handles
batch b=c//4 and heads [8g, 8g+8) where g=c%4 (kv heads [2g, 2g+2)).
Each core computes a partial transposed output [D, T] = o_proj_cols @ ctx;
the host sums the 4 partials per batch and transposes (all-reduce host-side).

v2 design notes (from perfetto trace of the fp32r baseline: PE stream 255us
but PE-array occupancy 437us -> ~180us of unhidden stationary loads, phase-2
at 35% stream efficiency, phases fully serial):
  - everything bf16 (same 1 cycle/row PE stream rate as fp32r, but half the
    stationary-load time, half the DMA bytes, no moving>=256 restriction so
    the causal diagonal is trimmed to 128-col granularity).
  - phase 1 streams 1024-token blocks per weight load (16-matmul PSUM
    accumulation chains hide the loads entirely).
  - phase 2 keeps keys-stationary chunks but emits the two per-chunk ctx
    matmuls as one strided-AP matmul, exp as one strided ACT op.
  - phase 3 computes out.T (o_proj cols stationary, ctx moving) so each
    stationary load streams 1024 tokens; host transposes.
  - phases interleaved for engine overlap: phase-2 rounds for query blocks
    a=0,1 run between phase-1 groups (ACT exp overlaps PE projections);
    phase-1 second half runs inside the a=0,1 round stream; phase-3 first
    token half is interleaved into the a=2,3 rounds (fills PE while ACT
    does exp); only the second token half of phase 3 trails.
  - v transposed via XBAR dma_start_transpose (no PE/PSUM involvement).
"""

import os
import sys

sys.path.insert(0, "/opt/trn_rl_repo")

import numpy as np

import concourse.bass as bass
import concourse.tile as tile
from concourse import bacc, mybir
from concourse.bass_utils import run_bass_kernel_spmd

F32 = mybir.dt.float32
BF16 = mybir.dt.bfloat16
EXP = mybir.ActivationFunctionType.Exp

B, T, D = 2, 2048, 2048
HQ, HK = 32, 8
DH = D // HQ              # 64 head dim
N_CORES = 8
GROUPS = 4                # head groups per batch
QCOLS = D // GROUPS       # 512 q cols per core
KCOLS = (D // 4) // GROUPS  # 128 k cols per core (2 kv heads)
WCOLS = QCOLS + 2 * KCOLS   # 768
KT = D // 128             # 16 contraction tiles
TB = 1024                 # phase-1/3 token block (half of T)
NEG = -480.0              # additive mask pre-scale (-60 after 1/8 scale)

_cache = {}


def _build():
    nc = bacc.Bacc("TRN2", target_bir_lowering=False, debug=False)

    xT_d = nc.declare_dram_parameter("xT", [D, T], BF16, isOutput=False)
    wqkv_d = nc.declare_dram_parameter("wqkv", [D, WCOLS], BF16, isOutput=False)
    oproj_d = nc.declare_dram_parameter("oproj", [128, 4, D], BF16, isOutput=False)
    masks_d = nc.declare_dram_parameter("masks", [128, 2, 128], BF16, isOutput=False)
    ones_d = nc.declare_dram_parameter("ones", [128, 16 * 80], BF16, isOutput=False)
    out_d = nc.declare_dram_parameter("out", [D, T], BF16, isOutput=True)
    rcscr_d = nc.dram_tensor("rcscratch", [16, 1024], F32)

    with tile.TileContext(nc) as tc:
        with (
            tc.tile_pool(name="pers", bufs=1) as pers,
            tc.tile_pool(name="xt", bufs=20) as xtp,
            tc.tile_pool(name="ep", bufs=4) as epool,
            tc.tile_pool(name="work", bufs=2) as work,
            tc.tile_pool(name="psum", bufs=1, space="PSUM") as psum,
        ):
            # ---- persistent SBUF ----
            wqkv_sb = pers.tile([128, KT, WCOLS], BF16, tag="wqkv")
            oproj_sb = pers.tile([128, 4, D], BF16, tag="oproj")
            masks_sb = pers.tile([128, 2, 128], BF16, tag="masks")
            qt = [pers.tile([128, T], BF16, tag=f"qt{m}", name=f"qt{m}")
                  for m in range(4)]
            kp = [pers.tile([128, T], BF16, tag=f"kp{k}", name=f"kp{k}")
                  for k in range(2)]
            vT = pers.tile([128, T], BF16, tag="vT")
            vs = [pers.tile([128, 16 * 80], BF16, tag=f"vs{k}", name=f"vs{k}")
                  for k in range(2)]
            ctx = [pers.tile([128, T], BF16, tag=f"ctx{m}", name=f"ctx{m}")
                   for m in range(4)]

            # ---- weight/constant DMAs (w[k] just ahead of x[k]) ----
            wq_r = wqkv_d[:].rearrange("(k p) c -> p k c", p=128)
            xts = {}

            def load_x_half(half, with_w=False):
                ts = slice(TB * half, TB * half + TB)
                tiles = []
                for k in range(KT):
                    if with_w:
                        nc.sync.dma_start(wqkv_sb[:, k, :], wq_r[:, k, :])
                    xt = xtp.tile([128, TB], BF16, tag="xt", bufs=20, name="xt")
                    nc.sync.dma_start(xt, xT_d[128 * k: 128 * k + 128, ts])
                    tiles.append(xt)
                    if with_w and k == 3:
                        nc.sync.dma_start(masks_sb, masks_d[:])
                        nc.sync.dma_start(vs[0], ones_d[:])
                        nc.sync.dma_start(vs[1], ones_d[:])
                xts[half] = tiles

            def v2(ap):
                return ap.rearrange("p (h q) -> p h q", h=2)

            # ---- phase 1: one 16-matmul chain per (m, token-half) ----
            def p1_group(mi, half, eng):
                ts = slice(TB * half, TB * half + TB)
                ps = psum.tile([128, TB], F32, tag="s2", bufs=3, name="p1ps")
                # matmul out must stay within one PSUM bank (512 f32 cols)
                for sub in (0, 512):
                    for k in range(KT):
                        nc.tensor.matmul(
                            ps[:, sub: sub + 512],
                            wqkv_sb[:, k, 128 * mi: 128 * mi + 128],
                            xts[half][k][:, sub: sub + 512],
                            start=(k == 0),
                            stop=(k == KT - 1),
                        )
                copy = eng.copy if eng is nc.scalar else eng.tensor_copy
                if mi < 4:
                    copy(qt[mi][:, ts], ps)
                elif mi == 4:
                    copy(kp[0][0:64, ts], ps[0:64, :])
                    copy(kp[1][64:128, ts], ps[64:128, :])
                    # kT duplicates at the other partition half
                    nc.sync.dma_start(kp[0][64:128, ts], kp[0][0:64, ts])
                    nc.sync.dma_start(kp[1][0:64, ts], kp[1][64:128, ts])
                else:
                    copy(vT[:, ts], ps)
                    # v chunks transposed into vs via XBAR dma (ones col 64
                    # of each 65-block left intact -> softmax denominator)
                    for kv in range(2):
                        vsr = vs[kv][:].rearrange("p (c e) -> p c e", e=80)
                        for c in range(8):
                            cc = 8 * half + c
                            nc.sync.dma_start_transpose(
                                vsr[:, cc, 0:64],
                                vT[64 * kv: 64 * kv + 64,
                                   128 * cc: 128 * cc + 128],
                            )

            # ---- phase 2: attention round per (head pair m, 512-query a) ----
            pending_norm = []

            def flush_norm(keep=0):
                while len(pending_norm) > keep:
                    pending_norm.pop(0)()

            def p2_round(m, a):
                kv = m // 2
                nj = 4 * (a + 1)
                qb = 512 * a
                ctxAB = psum.tile([65, 1024], F32, tag="ctx", bufs=1,
                                  name="ctxAB")
                pend = []

                def drain(last):
                    pE, pjc, plo = pend.pop(0)
                    for h2 in range(2):
                        nc.tensor.matmul(
                            ctxAB[:, 512 * h2 + plo: 512 * h2 + 512],
                            vs[kv][:, 80 * pjc: 80 * pjc + 65],
                            pE[:, 512 * h2 + plo: 512 * h2 + 512],
                            start=(pjc == 0),
                            stop=last,
                        )

                for jc in range(nj):
                    o = jc - 4 * a
                    lo = (0, 128, 256, 384)[o] if o >= 0 else 0
                    if jc == 2:
                        flush_norm(1)
                    S = psum.tile([128, 1024], F32, tag="s2", bufs=3, name="S")
                    for h2 in range(2):
                        nc.tensor.matmul(
                            S[:, 512 * h2 + lo: 512 * h2 + 512],
                            kp[kv][64 * h2: 64 * h2 + 64,
                                   128 * jc: 128 * jc + 128],
                            qt[m][64 * h2: 64 * h2 + 64, qb + lo: qb + 512],
                            start=True,
                            stop=True,
                            tile_position=(64 * h2, 0),
                        )
                    if o >= 0:
                        nc.vector.tensor_add(
                            v2(S)[:, :, lo: lo + 128],
                            v2(S)[:, :, lo: lo + 128],
                            masks_sb,
                        )
                    E = epool.tile([128, 1024], BF16, tag="E", bufs=4, name="E")
                    if lo == 0:
                        nc.scalar.activation(E, S, EXP, scale=0.125)
                    else:
                        nc.scalar.activation(
                            v2(E)[:, :, lo:512], v2(S)[:, :, lo:512],
                            EXP, scale=0.125,
                        )
                    pend.append((E, jc, lo))
                    if len(pend) > 2:
                        drain(False)
                while pend:
                    drain(len(pend) == 1)

                # denominator path starts immediately (DMA/rcp only);
                # the DVE multiplies are deferred one round so they never
                # stall this round's pipeline waiting on the DRAM bounce.
                cu = work.tile([65, 1024], F32, tag="cu", bufs=3, name="cu")
                nc.vector.tensor_copy(cu, ctxAB)
                den128 = work.tile([128, 8], F32, tag="d128", bufs=2,
                                   name="den128")
                nc.sync.dma_start(den128, cu[64:65, :])
                rcp = work.tile([128, 8], F32, tag="rcp", bufs=2, name="rcp")
                nc.vector.reciprocal(rcp, den128)
                ma = m * 4 + a
                nc.sync.dma_start(rcscr_d[ma: ma + 1, :], rcp)
                bcs = work.tile([64, 1024], F32, tag="bcs", bufs=3,
                                name="bcs")
                nc.sync.dma_start(
                    bcs, rcscr_d[ma: ma + 1, :].partition_broadcast(64)
                )

                def _norm(cu=cu, bcs=bcs, m=m, a=a):
                    isl = slice(512 * a, 512 * a + 512)
                    nc.vector.tensor_mul(
                        ctx[m][0:64, isl], cu[0:64, 0:512], bcs[:, 0:512]
                    )
                    tmpB = work.tile([64, 512], BF16, tag="tb", bufs=2,
                                     name="tmpB")
                    nc.vector.tensor_mul(
                        tmpB, cu[0:64, 512:1024], bcs[:, 512:1024]
                    )
                    nc.sync.dma_start(ctx[m][64:128, isl], tmpB)

                pending_norm.append(_norm)

            # ---- phase 3: out.T group per (128-outcol chunk, token half) ----
            def p3_group(rc, th, eng=None):
                eng = eng or nc.vector
                tsl = slice(TB * th, TB * th + TB)
                ps3 = psum.tile([128, TB], F32, tag="s2", bufs=3, name="p3ps")
                for sub in (0, 512):
                    for m in range(4):
                        nc.tensor.matmul(
                            ps3[:, sub: sub + 512],
                            oproj_sb[:, m, 128 * rc: 128 * rc + 128],
                            ctx[m][:, TB * th + sub: TB * th + sub + 512],
                            start=(m == 0),
                            stop=(m == 3),
                        )
                ostage = work.tile([128, TB], BF16, tag="ostage", bufs=3,
                                   name="ostage")
                if eng is nc.scalar:
                    eng.copy(ostage, ps3)
                else:
                    eng.tensor_copy(ostage, ps3)
                nc.sync.dma_start(out_d[128 * rc: 128 * rc + 128, tsl], ostage)

            # ================= emission =================
            load_x_half(0, with_w=True)
            p1_group(4, 0, nc.scalar)
            p1_group(5, 0, nc.scalar)
            p1_group(0, 0, nc.vector)
            p2_round(0, 0)
            p1_group(1, 0, nc.scalar)
            p2_round(1, 0)
            p1_group(2, 0, nc.vector)
            p2_round(2, 0)
            p1_group(3, 0, nc.scalar)
            p2_round(3, 0)

            nc.sync.dma_start(oproj_sb, oproj_d[:])
            load_x_half(1)
            p1_group(4, 1, nc.vector)
            p2_round(0, 1)
            p1_group(5, 1, nc.vector)
            p2_round(1, 1)
            p1_group(0, 1, nc.vector)
            p2_round(2, 1)
            p1_group(1, 1, nc.vector)
            p2_round(3, 1)
            p1_group(2, 1, nc.vector)
            p1_group(3, 1, nc.vector)

            flush_norm(0)
            p3c = 0
            for a in (2, 3):
                for m in range(4):
                    p2_round(m, a)
                    p3_group(p3c, 0)
                    p3c += 1
                    p3_group(p3c, 0)
                    p3c += 1
            flush_norm(0)
            for rc in range(16):
                p3_group(rc, 1, nc.scalar if rc % 2 == 0 else nc.vector)

    nc.compile()
    return nc


def _host_inputs(x, q_proj, k_proj, v_proj, o_proj):
    """Per-core input dicts (numpy, bf16)."""
    import ml_dtypes
    bf = ml_dtypes.bfloat16

    jj = np.arange(128)[:, None]
    cc = np.arange(128)[None, :]
    tri = np.where(jj <= cc, 0.0, NEG).astype(np.float32)
    masks = np.stack([tri, tri], axis=1).astype(bf)  # [128, 2, 128]
    ones = np.ones((128, 16 * 80), dtype=np.float32).astype(bf)

    xT = [np.ascontiguousarray(x[b].T).astype(bf) for b in range(B)]
    in_maps = []
    for c in range(N_CORES):
        b, g = divmod(c, GROUPS)
        wqkv = np.concatenate(
            [
                q_proj[QCOLS * g: QCOLS * g + QCOLS].T,
                k_proj[KCOLS * g: KCOLS * g + KCOLS].T,
                v_proj[KCOLS * g: KCOLS * g + KCOLS].T,
            ],
            axis=1,
        ).astype(bf)
        op = o_proj[:, QCOLS * g: QCOLS * g + QCOLS].T  # [512 e, 2048 r]
        op = np.ascontiguousarray(
            op.reshape(4, 128, D).transpose(1, 0, 2)
        ).astype(bf)
        in_maps.append(
            {
                "xT": xT[b],
                "wqkv": np.ascontiguousarray(wqkv),
                "oproj": op,
                "masks": masks,
                "ones": ones,
            }
        )
    return in_maps


def run(x, q_proj, k_proj, v_proj, o_proj, trace=False):
    """Run on hardware; returns (output [B,T,D] f32, BassKernelResults)."""
    if "nc" not in _cache:
        _cache["nc"] = _build()
    nc = _cache["nc"]
    in_maps = _host_inputs(x, q_proj, k_proj, v_proj, o_proj)
    res = run_bass_kernel_spmd(
        nc, in_maps, core_ids=list(range(N_CORES)), trace=trace
    )
    parts = [res.results[c]["out"] for c in range(N_CORES)]
    out = np.empty((B, T, D), dtype=np.float32)
    for b in range(B):
        acc = parts[4 * b].astype(np.float64)
        for g in range(1, GROUPS):
            acc += parts[4 * b + g].astype(np.float64)
        out[b] = acc.T.astype(np.float32)
    return out, res


def kernel(x, q_proj, k_proj, v_proj, o_proj, hq=None, hk=None, **_unused):
    x = np.asarray(x, dtype=np.float32)
    q_proj = np.asarray(q_proj, dtype=np.float32)
    k_proj = np.asarray(k_proj, dtype=np.float32)
    v_proj = np.asarray(v_proj, dtype=np.float32)
    o_proj = np.asarray(o_proj, dtype=np.float32)
    assert x.shape == (B, T, D), x.shape
    trace = bool(os.environ.get("KERNEL_TRACE"))
    out, _ = run(x, q_proj, k_proj, v_proj, o_proj, trace=trace)
    return out
